# revision 2
# baseline (speedup 1.0000x reference)
"""Trainium2 Bass kernel for nn_Crop (per-row random crop of audio).

Reference semantics:
    out[i, j] = audio[i, j]             for j <  starts[i]
    out[i, j] = audio[i, j + CROP_NUM]  for j >= starts[i]

Strategy (pure data parallel, 16 rows per core across 8 cores):
Each output row is two contiguous copies of the source row with a
data-dependent split point.  We view each output row as 128 blocks of
W=2048 elements (L = 128*W exactly).  Two indirect DMA gathers with
host-computed per-lane block indices move each block exactly once:
  - "shifted" gather (element_offset=CROP) for lanes >= boundary lane
  - "identity" gather (element_offset=0)  for lanes <= boundary lane
Invalid lanes carry an out-of-range index and are dropped by the DMA
bounds check (no bytes moved), which is how the per-row dynamic sizes
are expressed in a single static SPMD program.  The one block that
straddles starts[i] is fixed element-exactly with a mask compare +
predicated copy before a plain static store.  HBM traffic is therefore
~(read OUT_LEN + write OUT_LEN) per row == the memory roofline.
"""

import numpy as np

import concourse.bacc as bacc
import concourse.bass as bass
import concourse.mybir as mybir
from concourse import bass_utils
from concourse.bass import IndirectOffsetOnAxis
from concourse.tile import TileContext

# Problem constants (hardcoded per harness contract).
B = 128
L = 262144
CROP = 26214
OUT_LEN = L - CROP  # 235930
N_CORES = 8
R = B // N_CORES  # 16 rows per core

W = 2048                      # block width; L == 128 * W
NP = 128                      # SBUF partitions == blocks per row
N_FULL = OUT_LEN // W         # 115 full output blocks
TAIL = OUT_LEN - N_FULL * W   # 410
N_BLK = N_FULL + 1            # 116 blocks cover the valid output row
SENTINEL = 100_000            # dropped-lane index (> bounds_check)
BOUNDS = (R + 1) * NP - 1     # any real block index of the padded shard is valid

_cached = None


def _build_program():
    """Build the single SPMD Bass/Tile program (shared by all 8 cores)."""
    nc = bacc.Bacc("TRN2", target_bir_lowering=False, debug=False)

    audio_pad = nc.dram_tensor(
        "audio_pad", [(R + 1) * NP, W], mybir.dt.float32, kind="ExternalInput"
    ).ap()
    id_idx = nc.dram_tensor(
        "id_idx", [NP, R], mybir.dt.int32, kind="ExternalInput"
    ).ap()
    sh_idx = nc.dram_tensor(
        "sh_idx", [NP, R], mybir.dt.int32, kind="ExternalInput"
    ).ap()
    starts_b = nc.dram_tensor(
        "starts_b", [NP, R], mybir.dt.float32, kind="ExternalInput"
    ).ap()
    pos = nc.dram_tensor(
        "pos", [NP, W], mybir.dt.float32, kind="ExternalInput"
    ).ap()
    out = nc.dram_tensor(
        "out", [R, OUT_LEN], mybir.dt.float32, kind="ExternalOutput"
    ).ap()

    with TileContext(nc) as tc:
        with (
            tc.tile_pool(name="consts", bufs=1) as consts,
            tc.tile_pool(name="work", bufs=4) as work,
            tc.tile_pool(name="masks", bufs=4) as masks,
        ):
            id_idx_sb = consts.tile([NP, R], mybir.dt.int32)
            sh_idx_sb = consts.tile([NP, R], mybir.dt.int32)
            starts_sb = consts.tile([NP, R], mybir.dt.float32)
            pos_sb = consts.tile([NP, W], mybir.dt.float32)
            nc.sync.dma_start(out=id_idx_sb[:], in_=id_idx[:])
            nc.sync.dma_start(out=sh_idx_sb[:], in_=sh_idx[:])
            nc.sync.dma_start(out=starts_sb[:], in_=starts_b[:])
            nc.sync.dma_start(out=pos_sb[:], in_=pos[:])

            for i in range(R):
                t_out = work.tile([NP, W], mybir.dt.float32, tag="t_out")
                t_id = work.tile([NP, W], mybir.dt.float32, tag="t_id")
                mask = masks.tile([NP, W], mybir.dt.uint8, tag="mask")

                # Shifted blocks: lane k reads audio[i, k*W + CROP : ...].
                nc.gpsimd.indirect_dma_start(
                    out=t_out[:],
                    out_offset=None,
                    in_=audio_pad[:],
                    in_offset=IndirectOffsetOnAxis(ap=sh_idx_sb[:, i : i + 1], axis=0),
                    element_offset=CROP,
                    bounds_check=BOUNDS,
                    oob_is_err=False,
                )
                # Identity blocks: lane k reads audio[i, k*W : ...].
                nc.gpsimd.indirect_dma_start(
                    out=t_id[:],
                    out_offset=None,
                    in_=audio_pad[:],
                    in_offset=IndirectOffsetOnAxis(ap=id_idx_sb[:, i : i + 1], axis=0),
                    element_offset=0,
                    bounds_check=BOUNDS,
                    oob_is_err=False,
                )
                # mask[p, j] = (p*W + j) < starts[i]; exact split incl. boundary.
                nc.vector.tensor_scalar(
                    mask[:], pos_sb[:], starts_sb[:, i : i + 1], None,
                    mybir.AluOpType.is_lt,
                )
                nc.vector.copy_predicated(t_out[:], mask[:], t_id[:])

                dst_main = out[i, : N_FULL * W].rearrange("(p w) -> p w", w=W)
                nc.sync.dma_start(out=dst_main, in_=t_out[:N_FULL, :])
                dst_tail = out[i, N_FULL * W : OUT_LEN].rearrange(
                    "(p w) -> p w", w=TAIL
                )
                nc.sync.dma_start(out=dst_tail, in_=t_out[N_FULL : N_FULL + 1, :TAIL])

    nc.compile()
    return nc


def _host_inputs(audio: np.ndarray, starts: np.ndarray):
    """Shard + build per-core metadata inputs."""
    audio = np.ascontiguousarray(audio, dtype=np.float32)
    starts = np.asarray(starts, dtype=np.int32)

    lane = np.arange(NP, dtype=np.int32)  # [128]
    pos = (lane[:, None] * W + np.arange(W, dtype=np.int32)[None, :]).astype(
        np.float32
    )

    in_maps = []
    for c in range(N_CORES):
        rows = slice(c * R, (c + 1) * R)
        a = audio[rows].reshape(-1)
        a_pad = np.concatenate([a, np.zeros(NP * W, dtype=np.float32)])
        a_pad = a_pad.reshape((R + 1) * NP, W)

        s = starts[rows]  # [R]
        p_star = s // W  # boundary lane per row, [R]

        # lane k, row i
        base = (np.arange(R, dtype=np.int32) * NP)[None, :] + lane[:, None]  # [NP,R]
        id_idx = np.where(lane[:, None] <= p_star[None, :], base, SENTINEL)
        sh_idx = np.where(
            (lane[:, None] >= p_star[None, :]) & (lane[:, None] < N_BLK),
            base,
            SENTINEL,
        )
        starts_bcast = np.broadcast_to(
            s[None, :].astype(np.float32), (NP, R)
        ).copy()

        in_maps.append(
            {
                "audio_pad": np.ascontiguousarray(a_pad),
                "id_idx": np.ascontiguousarray(id_idx.astype(np.int32)),
                "sh_idx": np.ascontiguousarray(sh_idx.astype(np.int32)),
                "starts_b": starts_bcast,
                "pos": pos,
            }
        )
    return in_maps


def kernel(audio: np.ndarray, starts: np.ndarray) -> np.ndarray:
    global _cached
    if _cached is None:
        _cached = _build_program()
    nc = _cached

    in_maps = _host_inputs(audio, starts)
    res = bass_utils.run_bass_kernel_spmd(
        nc, in_maps, core_ids=list(range(N_CORES))
    )
    kernel.last_results = res
    out = np.concatenate([r["out"] for r in res.results], axis=0)
    return out.astype(np.float32)


def run_traced(audio: np.ndarray, starts: np.ndarray):
    """Run with NTFF tracing enabled; returns BassKernelResults."""
    global _cached
    if _cached is None:
        _cached = _build_program()
    in_maps = _host_inputs(audio, starts)
    return bass_utils.run_bass_kernel_spmd(
        _cached, in_maps, core_ids=list(range(N_CORES)), trace=True
    )
